# revision 9
# baseline (speedup 1.0000x reference)
"""Correlation layer (epipolar-masked, column-normalized) on 8 TRN2 cores.

Per batch b (one NeuronCore each):
  corr[q,k] = <fB[q,:], fA[k,:]>            (q,k in [0,4096), c=256)
  masked    = corr * W[q,k]
  s[k]      = sum_q masked[q,k]
  out[q,k]  = masked[q,k] / (s[k] + 1e-8)

Kernel structure per core: loop over 8 k-blocks of 512 columns.
  PE    : 32 fp32 matmul pairs per block (c contraction split 2x128),
          plus per-block ones-matmul partition reduce of the accumulated
          column sums and a K=1 broadcast matmul of 1/(s+eps).
  DVE   : mask-multiply PSUM corr x W tile -> SBUF slab; rs = 1/(s+eps);
          final scale of the slab by rs (broadcast access pattern).
  GPSIMD: accumulates column sums across the 32 q-tiles of a block.
  ACT   : issues output DMAs (second HWDGE queue).
  SYNC  : input DMAs (features once, W streamed, exactly one pass).

All engines run fully-unrolled instruction streams with explicit
semaphores (raw bass; the Tile framework's auto-sync emits multi-wait
instructions this compiler build rejects).
"""
import numpy as np

import concourse.bass as bass
import concourse.mybir as mybir
from concourse.bass_utils import run_bass_kernel_spmd

F32 = mybir.dt.float32

B = 8
HW = 4096          # 64*64 pixels
C = 256            # channels
CH = C // 128      # c halves (2)
KB = 8             # k blocks
KW = 512           # k block width
QT = HW // 128     # q tiles per block (32)
NPS = 6            # corr PSUM buffers
NWB = 8            # W SBUF ring buffers
TAIL_LAG = 4       # pairs into next block before prev block's s-matmul
NCH = 4            # scale/store chunks per block (8 q-tiles each)
CHQ = QT // NCH    # q-tiles per chunk


def build_nc(no_norm=False):
    nc = bass.Bass()

    fBT_d = nc.dram_tensor("fBT", [CH, 128, HW], F32, kind="ExternalInput")
    fAT_d = nc.dram_tensor("fAT", [CH, 128, HW], F32, kind="ExternalInput")
    W_d = nc.dram_tensor("W", [HW, HW], F32, kind="ExternalInput")
    ones_col_d = nc.dram_tensor("ones_col", [128, 1], F32, kind="ExternalInput")
    ones_row_d = nc.dram_tensor("ones_row", [1, 128], F32, kind="ExternalInput")
    out_d = nc.dram_tensor("out", [HW, HW], F32, kind="ExternalOutput")

    corr_ps = [
        nc.ctx.enter_context(nc.psum_tensor(f"corr{i}", [128, KW], F32))
        for i in range(NPS)
    ]

    # Per-ring-slot DMA-completion semaphores: HWDGE completions across the
    # 16 hardware sub-queues are NOT in issue order, so a single cumulative
    # counter cannot tell WHICH dma finished. One sem per W-ring slot and
    # per output chunk slot makes every wait track exactly its own slot.
    w_sems = [
        nc.ctx.enter_context(nc.semaphore(f"w_sem{i}")) for i in range(NWB)
    ]
    out_sems = [
        nc.ctx.enter_context(nc.semaphore(f"out_sem{i}")) for i in range(NCH)
    ]

    with (
        nc.sbuf_tensor("fBT_s", [128, CH * HW], F32) as fBT_s,
        nc.sbuf_tensor("fAT_s", [128, CH * HW], F32) as fAT_s,
        nc.sbuf_tensor("W_s", [128, NWB * KW], F32) as W_s,
        nc.sbuf_tensor("slab", [128, QT * KW], F32) as slab,
        nc.sbuf_tensor("acc", [128, KW], F32) as acc,
        nc.sbuf_tensor("rs_s", [1, KW], F32) as rs_s,
        nc.sbuf_tensor("ones_col_s", [128, 1], F32) as ones_col,
        nc.sbuf_tensor("ones_row_s", [1, 128], F32) as ones_row,
        nc.psum_tensor([1, KW], F32) as s_ps,
        nc.psum_tensor([128, KW], F32) as bc_ps,
        nc.semaphore("fin_sem") as fin_sem,
        nc.semaphore("pe_sem") as pe_sem,
        nc.semaphore("mask_sem") as mask_sem,
        nc.semaphore("acc_sem") as acc_sem,
        nc.semaphore("pe_s_sem") as pe_s_sem,
        nc.semaphore("rs_sem") as rs_sem,
        nc.semaphore("pe_b_sem") as pe_b_sem,
        nc.semaphore("scale_sem") as scale_sem,
        nc.Block() as block,
    ):
        out_view = out_d[:].rearrange("(t p) k -> p t k", p=128)
        slab3 = slab[:].rearrange("p (t n) -> p t n", n=KW)
        bc_view = (bc_ps[:].rearrange("p (o n) -> p o n", o=1)
                   .broadcast_to([128, CHQ, KW]))

        @block.sync
        def _(sync):
            for h in range(CH):
                sync.dma_start(
                    fBT_s[:, h * HW:(h + 1) * HW], fBT_d[h]
                ).then_inc(fin_sem, 16)
                sync.dma_start(
                    fAT_s[:, h * HW:(h + 1) * HW], fAT_d[h]
                ).then_inc(fin_sem, 16)
            sync.dma_start(ones_col[:], ones_col_d[:]).then_inc(fin_sem, 16)
            sync.dma_start(ones_row[:], ones_row_d[:]).then_inc(fin_sem, 16)
            for kb in range(KB):
                for j in range(QT):
                    idx = kb * QT + j
                    if idx >= NWB:
                        sync.wait_ge(mask_sem, idx - NWB + 1)
                    wb = idx % NWB
                    sync.dma_start(
                        W_s[:, wb * KW:(wb + 1) * KW],
                        W_d[j * 128:(j + 1) * 128, kb * KW:(kb + 1) * KW],
                    ).then_inc(w_sems[wb], 16)
            for i in range(NWB):
                sync.wait_ge(w_sems[i], 16 * (KB * QT // NWB))

        @block.tensor
        def _(tensor):
            def s_mm(kb):
                tensor.wait_ge(acc_sem, QT * (kb + 1))
                nc.tensor.matmul(
                    s_ps[:], ones_col[:], acc[:], start=True, stop=True
                ).then_inc(pe_s_sem, 1)

            def bc_mm(kb):
                tensor.wait_ge(rs_sem, kb + 1)
                if kb > 0:
                    tensor.wait_ge(scale_sem, NCH * kb)
                nc.tensor.matmul(
                    bc_ps[:], ones_row[:], rs_s[:], start=True, stop=True
                ).then_inc(pe_b_sem, 1)

            tensor.wait_ge(fin_sem, 16 * (2 * CH + 2))
            for kb in range(KB):
                for j in range(QT):
                    idx = kb * QT + j
                    if kb > 0 and j == TAIL_LAG:
                        s_mm(kb - 1)
                    if kb > 0 and j == TAIL_LAG + 2:
                        bc_mm(kb - 1)
                    if idx >= NPS:
                        tensor.wait_ge(mask_sem, idx - NPS + 1)
                    ps = corr_ps[idx % NPS]
                    qs = slice(j * 128, (j + 1) * 128)
                    ks = slice(kb * KW, (kb + 1) * KW)
                    nc.tensor.matmul(
                        ps[:], fBT_s[:, qs], fAT_s[:, ks],
                        start=True, stop=False,
                    )
                    nc.tensor.matmul(
                        ps[:],
                        fBT_s[:, HW + j * 128:HW + (j + 1) * 128],
                        fAT_s[:, HW + kb * KW:HW + (kb + 1) * KW],
                        start=False, stop=True,
                    ).then_inc(pe_sem, 1)
            s_mm(KB - 1)
            bc_mm(KB - 1)

        @block.vector
        def _(vector):
            for kb in range(KB):
                for j in range(QT):
                    idx = kb * QT + j
                    vector.wait_ge(pe_sem, idx + 1)
                    wb = idx % NWB
                    vector.wait_ge(w_sems[wb], 16 * (idx // NWB + 1))
                    if kb > 0:
                        ch = j // CHQ
                        vector.wait_ge(out_sems[ch], 16 * kb)
                    nc.vector.tensor_mul(
                        slab[:, j * KW:(j + 1) * KW],
                        corr_ps[idx % NPS][:],
                        W_s[:, wb * KW:(wb + 1) * KW],
                    ).then_inc(mask_sem, 1)
                # block tail: rs = 1/(s + eps), then scale chunks
                vector.wait_ge(pe_s_sem, kb + 1)
                nc.vector.tensor_scalar_add(rs_s[:], s_ps[:], 1e-8)
                nc.vector.reciprocal(rs_s[:], rs_s[:]).then_inc(rs_sem, 1)
                vector.wait_ge(pe_b_sem, kb + 1)
                for ch in range(NCH):
                    ts = slice(ch * CHQ, (ch + 1) * CHQ)
                    if no_norm:
                        # debug: pass masked values through unscaled
                        nc.vector.tensor_scalar_add(
                            slab3[:, ts, :], slab3[:, ts, :], 0.0
                        ).then_inc(scale_sem, 1)
                    else:
                        nc.vector.tensor_mul(
                            slab3[:, ts, :], slab3[:, ts, :], bc_view
                        ).then_inc(scale_sem, 1)

        @block.gpsimd
        def _(gpsimd):
            for kb in range(KB):
                for j in range(QT):
                    idx = kb * QT + j
                    gpsimd.wait_ge(mask_sem, idx + 1)
                    src = slab[:, j * KW:(j + 1) * KW]
                    if j == 0:
                        if kb > 0:
                            gpsimd.wait_ge(pe_s_sem, kb)
                        nc.gpsimd.tensor_copy(acc[:], src).then_inc(acc_sem, 1)
                    else:
                        nc.gpsimd.tensor_add(acc[:], acc[:], src).then_inc(
                            acc_sem, 1
                        )

        @block.scalar
        def _(scalar):
            for kb in range(KB):
                for ch in range(NCH):
                    scalar.wait_ge(scale_sem, NCH * kb + ch + 1)
                    ts = slice(ch * CHQ, (ch + 1) * CHQ)
                    scalar.dma_start(
                        out_view[:, ts, kb * KW:(kb + 1) * KW],
                        slab3[:, ts, :],
                    ).then_inc(out_sems[ch], 16)
            for i in range(NCH):
                scalar.wait_ge(out_sems[i], 16 * KB)

    return nc


_CACHED_NC = None


def _get_nc():
    global _CACHED_NC
    if _CACHED_NC is None:
        _CACHED_NC = build_nc()
    return _CACHED_NC


def _prep_inputs(feature_A, feature_B, Weight):
    feature_A = np.ascontiguousarray(feature_A, dtype=np.float32)
    feature_B = np.ascontiguousarray(feature_B, dtype=np.float32)
    W_flat = np.ascontiguousarray(Weight, dtype=np.float32).reshape(HW, HW)
    ones_col = np.ones((128, 1), dtype=np.float32)
    ones_row = np.ones((1, 128), dtype=np.float32)
    in_maps = []
    for b in range(B):
        fA = feature_A[b].reshape(HW, C)
        fB = feature_B[b].reshape(HW, C)
        fAT = np.ascontiguousarray(fA.T).reshape(CH, 128, HW)
        fBT = np.ascontiguousarray(fB.T).reshape(CH, 128, HW)
        in_maps.append({
            "fBT": fBT,
            "fAT": fAT,
            "W": W_flat,
            "ones_col": ones_col,
            "ones_row": ones_row,
        })
    return in_maps


def kernel(feature_A, feature_B, Weight):
    nc = _get_nc()
    in_maps = _prep_inputs(feature_A, feature_B, Weight)
    res = run_bass_kernel_spmd(nc, in_maps, core_ids=list(range(B)))
    out = np.stack([res.results[b]["out"] for b in range(B)], axis=0)
    return out.reshape(B, 64, 64, HW)
